# revision 4
# baseline (speedup 1.0000x reference)
"""DeepRC segment-softmax attention pooling kernel for 8 Trainium2 NeuronCores.

Strategy (bag-sharded, zero collectives):
  - segment_ids is sorted with 8 bags; core b gets exactly bag b's instance
    range (host computes boundaries with searchsorted), padded to a common
    NPAD (multiple of 2048) with zeros + a 0/1 mask.
  - Host pre-transposes each core's x slice to [(l,c)=736 rows (padded 768),
    NPAD cols] so the conv contraction dim lies on SBUF partitions and all
    DMA loads are fully coalesced.
  - Conv1d(K=32,C=23,KS=9,L=32->24) is a banded matmul: W2T[(l',c),(l,k)]
    = w[k,c,l'-l].  Rows fold into 6 x 128-partition blocks, cols into
    6 M-blocks (4 l-values x 32 k each); 20 (s,t) blocks are nonzero.
  - SELU is monotone => maxpool-over-l commutes before SELU; maxpool is a
    free-axis reduce over the 6 PSUM M-blocks + a partition fold (4->1
    groups of 32) done with shifted single-input copies + tensor_max.
  - SELU(y+b) = lam*relu(y+b) + min(exp(y+b+ln(lam*alpha)), lam*alpha) - lam*alpha;
    the trailing constant is folded into the next layer's bias / host output.
  - 4 subtiles of 512 instances stack to [128,512] so the attention MLP runs
    as block-diagonal matmuls at full partition occupancy.
  - Per (macrotile, subtile) softmax stats (m, z, pooled) accumulate online;
    host combines them exactly (float64) and applies the output head.
"""

import os
import sys

for _p in (
    "/root/.axon_site",
    "/root/.axon_site/_ro/trn_rl_repo",
    "/root/.axon_site/_ro/pypackages",
    "/opt/trn_rl_repo",
):
    if os.path.isdir(_p) and _p not in sys.path:
        sys.path.append(_p)

import numpy as np

import concourse.bass as bass
import concourse.mybir as mybir
from concourse.tile import TileContext, ScopedClock
from concourse.bass_utils import run_bass_kernel_spmd

AF = mybir.ActivationFunctionType
OP = mybir.AluOpType
AX = mybir.AxisListType
F32 = mybir.dt.float32

# ---------------------------------------------------------------- constants
N_BAGS = 8
L, C, K, U, KS = 32, 23, 32, 32, 9
LO = L - KS + 1            # 24 output positions
R = L * C                  # 736 rows of xT
RPAD = 768                 # 6 x 128
NT = 6                     # M blocks (each 4 l x 32 k)
FD = 512                   # instances per subtile (1 PSUM bank of f32)
QS = 4                     # subtiles stacked per macrotile
MACRO = QS * FD            # 2048

LAM = 1.0507009873554805
ALPHA = 1.6732632423543772
LA = LAM * ALPHA
LN_LA = float(np.log(LA))
C_SELU = -LA               # deferred selu constant

# ------------------------------------------------------- walrus workarounds


def _patched_drain_and_barrier(self, tick_clock, wait_clock):
    # stock version puts every outstanding sem wait on one drain; this
    # walrus build allows a single sync wait per instruction.
    nc = self.nc
    drain_inst = nc.sync.drain()
    wait_clock.add_sem_waits(
        drain_inst.ins, ScopedClock({None: tick_clock.global_clock})
    )
    si = drain_inst.ins.sync_info
    waits = list(si.on_wait or []) if si is not None else []
    if len(waits) > 1:
        si.on_wait = waits[:1]
        for w in waits[1:]:
            extra = nc.sync.drain()
            esi = extra.ins.sync_info
            if esi is None:
                extra.ins.sync_info = mybir.SyncInfo(on_wait=[w], on_update=[])
            else:
                esi.on_wait = [w]
    nc.all_engine_barrier()
    assert self.sems is not None
    popped = nc._tile_sem_poison_stack.pop()
    assert popped is self._sem_poison
    nc.clear_and_free_semaphores(list(self.sems.allocated().values()))
    nc.all_engine_barrier()


TileContext._drain_and_barrier = _patched_drain_and_barrier

_WSPLIT_CTR = [0]


def _split_multi_waits(nc):
    # move extra sem waits onto same-engine NoOps inserted just before the
    # owning instruction (equivalent gating, one wait per instruction).
    for func in nc.m.functions:
        for blk in func.blocks:
            out = []
            changed = False
            for inst in blk.instructions:
                si = inst.sync_info
                if si is not None and si.on_wait is not None and len(si.on_wait) > 1:
                    waits = list(si.on_wait)
                    for w in waits[:-1]:
                        _WSPLIT_CTR[0] += 1
                        nop = mybir.InstNoOp(
                            name=f"I-wsplit-{_WSPLIT_CTR[0]}", ins=[], outs=[]
                        )
                        nop.engine = inst.engine
                        nop.sync_info = mybir.SyncInfo(on_wait=[w], on_update=[])
                        out.append(nop)
                    si.on_wait = [waits[-1]]
                    changed = True
                out.append(inst)
            if changed:
                blk.instructions[:] = out
    return nc


# ------------------------------------------------------------- conv blocks


def _conv_block_list():
    """Nonzero (t, s) blocks of the banded weight matrix, t-major."""
    blocks = []
    for t in range(NT):
        lo_row = 23 * (4 * t)            # first needed row: l' = 4t
        hi_row = 23 * (4 * t + 12) + 22  # last needed row: l' = 4t+12, c=22
        s_lo, s_hi = lo_row // 128, hi_row // 128
        for s in range(s_lo, min(s_hi, 5) + 1):
            blocks.append((t, s))
    return blocks


CONV_BLOCKS = _conv_block_list()          # 20 blocks
N_CB = len(CONV_BLOCKS)


def _build_w2t(conv_w):
    w2t = np.zeros((RPAD, RPAD), np.float32)
    for l in range(LO):
        for j in range(KS):
            lp = l + j
            # rows 23*lp .. +23 ; cols 32*l .. +32 ; value w[k, c, j]
            w2t[23 * lp : 23 * lp + 23, 32 * l : 32 * l + 32] = conv_w[:, :, j].T
    return w2t


# --------------------------------------------------------------- program


def _build_program(NPAD):
    T = NPAD // MACRO
    nc = bass.Bass()
    xt_d = nc.declare_dram_parameter("xt", [RPAD, NPAD], F32, isOutput=False)
    wconv_d = nc.declare_dram_parameter("wconv", [128, N_CB * 128], F32, isOutput=False)
    wmisc_d = nc.declare_dram_parameter("wmisc", [128, 394], F32, isOutput=False)
    mask_d = nc.declare_dram_parameter("maskp", [QS, T * FD], F32, isOutput=False)
    m_out = nc.declare_dram_parameter("m_out", [QS, T], F32, isOutput=True)
    z_out = nc.declare_dram_parameter("z_out", [QS, T], F32, isOutput=True)
    pooled_out = nc.declare_dram_parameter("pooled_out", [128, T], F32, isOutput=True)

    with TileContext(nc) as tc:
        with (
            tc.tile_pool(name="wpool", bufs=1) as wpool,
            tc.tile_pool(name="xpool", bufs=3) as xpool,
            tc.tile_pool(name="spool", bufs=2) as spool,
            tc.tile_pool(name="cpsum", bufs=2, space="PSUM") as cpsum,
            tc.tile_pool(name="mpsum", bufs=2, space="PSUM") as mpsum,
        ):
            wsb = wpool.tile([128, N_CB * 128], F32)
            nc.sync.dma_start(wsb[:], wconv_d[:])
            wmisc = wpool.tile([128, 394], F32)
            nc.sync.dma_start(wmisc[:], wmisc_d[:])
            mask_sb = wpool.tile([QS, T * FD], F32)
            nc.sync.dma_start(mask_sb[:], mask_d[:])
            m_sb = wpool.tile([QS, T], F32)
            z_sb = wpool.tile([QS, T], F32)
            pooled_sb = wpool.tile([128, T], F32)

            w1bd = wmisc[:, 0:128]
            w2bd = wmisc[:, 128:256]
            w3bd = wmisc[:, 256:260]
            bc4 = wmisc[0:4, 260:388]
            be_exp = wmisc[:, 388:389]
            be_relu = wmisc[:, 389:390]
            bh1_exp = wmisc[:, 390:391]
            bh1_relu = wmisc[:, 391:392]
            bh2_exp = wmisc[:, 392:393]
            bh2_relu = wmisc[:, 393:394]

            for j in range(T):
                er4 = spool.tile([128, FD], F32, tag="er4")
                for q in range(QS):
                    col0 = j * MACRO + q * FD
                    xts = xpool.tile([128, NT, FD], F32, tag="xts")
                    for s in range(NT):
                        nc.sync.dma_start(
                            xts[:, s, :], xt_d[128 * s : 128 * (s + 1), col0 : col0 + FD]
                        )
                    # conv: two psum halves of 3 M-blocks each
                    halves = []
                    for h in range(2):
                        ps = cpsum.tile([128, 3, FD], F32, tag="cps")
                        for t in range(3 * h, 3 * h + 3):
                            slist = [s for (tt, s) in CONV_BLOCKS if tt == t]
                            for ki, s in enumerate(slist):
                                idx = CONV_BLOCKS.index((t, s))
                                nc.tensor.matmul(
                                    ps[:, t - 3 * h, :],
                                    wsb[:, idx * 128 : (idx + 1) * 128],
                                    xts[:, s, :],
                                    start=(ki == 0),
                                    stop=(ki == len(slist) - 1),
                                )
                        halves.append(ps)
                    # max over the 6 M-blocks (l-groups): 2 reduces + TT max
                    bmA = spool.tile([128, FD], F32, tag="bmA")
                    nc.vector.tensor_reduce(
                        bmA[:], halves[0][:].rearrange("p t f -> p f t"),
                        axis=AX.X, op=OP.max,
                    )
                    bmB = spool.tile([128, FD], F32, tag="bmB")
                    nc.vector.tensor_reduce(
                        bmB[:], halves[1][:].rearrange("p t f -> p f t"),
                        axis=AX.X, op=OP.max,
                    )
                    g = spool.tile([128, FD], F32, tag="g")
                    nc.vector.tensor_max(g[:], bmA[:], bmB[:])
                    # partition fold 128 -> 32 (max over 4 l-residue groups)
                    tmp64 = spool.tile([64, FD], F32, tag="tmp64")
                    nc.gpsimd.tensor_copy(tmp64[:], g[64:128, :])
                    f1 = spool.tile([64, FD], F32, tag="f1")
                    nc.vector.tensor_max(f1[:], g[0:64, :], tmp64[:])
                    tmp32 = spool.tile([32, FD], F32, tag="tmp32")
                    nc.gpsimd.tensor_copy(tmp32[:], f1[32:64, :])
                    nc.vector.tensor_max(
                        er4[32 * q : 32 * q + 32, :], f1[0:32, :], tmp32[:]
                    )

                # ---- selu(er4 + conv_b) (scaled branches, const deferred)
                t_relu = spool.tile([128, FD], F32, tag="t_relu")
                nc.scalar.activation(t_relu[:], er4[:], AF.Relu, bias=be_relu, scale=LAM)
                v_exp = spool.tile([128, FD], F32, tag="v_exp")
                nc.scalar.activation(v_exp[:], er4[:], AF.Exp, bias=be_exp, scale=1.0)
                e4 = spool.tile([128, FD], F32, tag="e4")
                nc.vector.scalar_tensor_tensor(
                    e4[:], v_exp[:], LA, t_relu[:], op0=OP.min, op1=OP.add
                )
                # ---- MLP layer 1
                ps1 = mpsum.tile([128, FD], F32, tag="mlp")
                nc.tensor.matmul(ps1[:], w1bd, e4[:])
                t1 = spool.tile([128, FD], F32, tag="t1")
                nc.scalar.activation(t1[:], ps1[:], AF.Relu, bias=bh1_relu, scale=LAM)
                v1 = spool.tile([128, FD], F32, tag="v1")
                nc.scalar.activation(v1[:], ps1[:], AF.Exp, bias=bh1_exp, scale=1.0)
                h1 = spool.tile([128, FD], F32, tag="h1")
                nc.vector.scalar_tensor_tensor(
                    h1[:], v1[:], LA, t1[:], op0=OP.min, op1=OP.add
                )
                # ---- MLP layer 2
                ps2 = mpsum.tile([128, FD], F32, tag="mlp")
                nc.tensor.matmul(ps2[:], w2bd, h1[:])
                t2 = spool.tile([128, FD], F32, tag="t2")
                nc.scalar.activation(t2[:], ps2[:], AF.Relu, bias=bh2_relu, scale=LAM)
                v2 = spool.tile([128, FD], F32, tag="v2")
                nc.scalar.activation(v2[:], ps2[:], AF.Exp, bias=bh2_exp, scale=1.0)
                h2 = spool.tile([128, FD], F32, tag="h2")
                nc.vector.scalar_tensor_tensor(
                    h2[:], v2[:], LA, t2[:], op0=OP.min, op1=OP.add
                )
                # ---- attention logits (bias b3 cancels in softmax)
                psa = mpsum.tile([4, FD], F32, tag="mlp")
                nc.tensor.matmul(psa[:], w3bd, h2[:])
                nc.vector.tensor_reduce(
                    m_sb[:, j : j + 1], psa[:], axis=AX.X, op=OP.max
                )
                negm = spool.tile([4, 1], F32, tag="negm")
                nc.vector.tensor_scalar_mul(negm[:], m_sb[:, j : j + 1], -1.0)
                pexp = spool.tile([4, FD], F32, tag="pexp")
                nc.scalar.activation(pexp[:], psa[:], AF.Exp, bias=negm[:], scale=1.0)
                p4 = spool.tile([4, FD], F32, tag="p4")
                nc.vector.tensor_mul(
                    p4[:], pexp[:], mask_sb[:, j * FD : (j + 1) * FD]
                )
                nc.vector.tensor_reduce(
                    z_sb[:, j : j + 1], p4[:], axis=AX.X, op=OP.add
                )
                # ---- pooled += e4 * broadcast(p) per subtile group
                psb = mpsum.tile([128, FD], F32, tag="mlp")
                nc.tensor.matmul(psb[:], bc4, p4[:])
                we = spool.tile([128, FD], F32, tag="we")
                nc.vector.tensor_mul(we[:], e4[:], psb[:])
                nc.vector.tensor_reduce(
                    pooled_sb[:, j : j + 1], we[:], axis=AX.X, op=OP.add
                )

            nc.sync.dma_start(m_out[:], m_sb[:])
            nc.sync.dma_start(z_out[:], z_sb[:])
            nc.sync.dma_start(pooled_out[:], pooled_sb[:])

    _split_multi_waits(nc)
    return nc


_PROGRAM_CACHE = {}
LAST_RESULTS = None  # set by kernel(); test.py reads trace/exec info


def _get_program(NPAD):
    if NPAD not in _PROGRAM_CACHE:
        _PROGRAM_CACHE[NPAD] = _build_program(NPAD)
    return _PROGRAM_CACHE[NPAD]


# ----------------------------------------------------------------- kernel


def kernel(
    inputs,
    segment_ids,
    conv_w,
    conv_b,
    att_w1,
    att_b1,
    att_w2,
    att_b2,
    att_w3,
    att_b3,
    out_w,
    out_b,
):
    global LAST_RESULTS
    x = np.asarray(inputs, np.float32)
    seg = np.asarray(segment_ids)
    conv_w = np.asarray(conv_w, np.float32)
    conv_b = np.asarray(conv_b, np.float32)
    att_w1 = np.asarray(att_w1, np.float32)
    att_b1 = np.asarray(att_b1, np.float32)
    att_w2 = np.asarray(att_w2, np.float32)
    att_b2 = np.asarray(att_b2, np.float32)
    att_w3 = np.asarray(att_w3, np.float32)
    att_b3 = np.asarray(att_b3, np.float32)
    out_w = np.asarray(out_w, np.float32)
    out_b = np.asarray(out_b, np.float32)

    n_total = x.shape[0]
    bounds = np.searchsorted(seg, np.arange(N_BAGS + 1))
    sizes = bounds[1:] - bounds[:-1]
    NPAD = max(MACRO, int(-(-sizes.max() // MACRO)) * MACRO)
    T = NPAD // MACRO

    # ---------------- weights (shared by all cores)
    w2t = _build_w2t(conv_w)
    wconv = np.zeros((128, N_CB * 128), np.float32)
    for idx, (t, s) in enumerate(CONV_BLOCKS):
        wconv[:, idx * 128 : (idx + 1) * 128] = w2t[
            128 * s : 128 * (s + 1), 128 * t : 128 * (t + 1)
        ]

    ones = np.ones(K, np.float32)
    b1p = att_b1 + C_SELU * (att_w1 @ np.ones(K, np.float32))
    b2p = att_b2 + C_SELU * (att_w2 @ np.ones(U, np.float32))

    wmisc = np.zeros((128, 394), np.float32)
    for q in range(QS):
        sl = slice(32 * q, 32 * q + 32)
        wmisc[sl, 0:128][:, sl] = att_w1.T          # w1bd
        wmisc[sl, 128:256][:, sl] = att_w2.T        # w2bd
        wmisc[sl, 256 + q] = att_w3[0]              # w3bd
        wmisc[q, 260 + 32 * q : 260 + 32 * q + 32] = 1.0  # bc4
        wmisc[sl, 388] = conv_b + LN_LA
        wmisc[sl, 389] = LAM * conv_b
        wmisc[sl, 390] = b1p + LN_LA
        wmisc[sl, 391] = LAM * b1p
        wmisc[sl, 392] = b2p + LN_LA
        wmisc[sl, 393] = LAM * b2p

    # ---------------- per-core inputs
    xf = x.reshape(n_total, R)
    in_maps = []
    for b in range(N_BAGS):
        s0, s1 = int(bounds[b]), int(bounds[b + 1])
        nb = s1 - s0
        xt = np.zeros((RPAD, NPAD), np.float32)
        xt[:R, :nb] = xf[s0:s1].T
        mask01 = np.zeros(NPAD, np.float32)
        mask01[:nb] = 1.0
        maskp = np.ascontiguousarray(
            mask01.reshape(T, QS, FD).transpose(1, 0, 2).reshape(QS, T * FD)
        )
        in_maps.append(
            {"xt": xt, "wconv": wconv, "wmisc": wmisc, "maskp": maskp}
        )

    nc = _get_program(NPAD)
    trace_mode = int(os.environ.get("DEEPRC_TRACE", "0"))
    kwargs = {}
    if trace_mode == 1:
        kwargs = dict(trace=True, trace_cores=[0])
    elif trace_mode >= 2:
        kwargs = dict(trace=True, trace_cores=list(range(N_BAGS)), stitch_traces=True)
    res = run_bass_kernel_spmd(
        nc,
        in_maps,
        core_ids=list(range(N_BAGS)),
        **kwargs,
    )
    LAST_RESULTS = res

    # ---------------- exact host combine (float64)
    out = np.zeros((N_BAGS, 1), np.float32)
    for b in range(N_BAGS):
        r = res.results[b]
        m = r["m_out"].astype(np.float64)          # [4, T]
        z = r["z_out"].astype(np.float64)          # [4, T]
        pooled = r["pooled_out"].astype(np.float64)  # [128, T]
        M = m.max()
        sc = np.exp(m - M)                          # [4, T]
        Z = (z * sc).sum()
        P = (pooled.reshape(QS, K, T) * sc[:, None, :]).sum(axis=(0, 2))  # [K]
        pooled_bag = P / Z + C_SELU
        out[b, 0] = np.float32(
            float(out_w.astype(np.float64)[0] @ pooled_bag) + float(out_b[0])
        )
    return out
